# revision 33
# baseline (speedup 1.0000x reference)
"""MoE adapter layer kernel for Trainium2 (8 NeuronCores, data-parallel over B).

Reference computation (per sample b):
    pooled = x[b].mean(axis=0)                       # (D,)
    gate   = softmax(pooled @ gate_w.T)              # (E,)
    top2 values/indices, renormalized weights w0,w1
    h_k    = gelu(x[b] @ Wd[ik].T + bd[ik])          # (S, BN)
    out[b] = sum_k w_k * h_k @ Wu[ik].T + sum_k w_k * bu[ik]

Shapes: B=32, S=2048, D=1024, BN=64, E=8, K=2.

Strategy: shard B over the 8 cores (4 samples each). The device runs the
heavy math -- per sample a (2048x1024)@(1024x128) down matmul, exact
GELU, and a (2048x128)@(128x1024) up matmul -- as one statically
scheduled fp16 pipeline at full PE rate with fp32 PSUM accumulation
(~7e-4 max-rel error). The streamed tensors (x in, out) travel as fp16,
halving HBM traffic vs fp32.

Routing runs on the host in exact fp32 (softmax over 8 gates + top-2 on
the seq-mean -- microseconds of numpy on <40KB of data): the host ships,
per sample, the two selected experts' down/up weights (up pre-scaled by
the renormalized routing weights) in the layouts the matmuls want, plus
the stacked down-bias row. This keeps the device free of dynamic
(register-indexed) DMAs, whose descriptor patching and serialized
trigger chain dominated the kernel's startup latency; every device DMA
is static and prefetchable. The down bias enters as one extra f32r
rank-1 matmul into the same PSUM accumulation group as the down matmul
(bias lives on partitions, expressible as bd_row^T @ ones); the out bias
(a per-sample constant row) is added during the host-side fp16->f32
conversion of the output.

Down matmul contracts D as 8 interleaved chunks (d = p*8+j) so x ships
transposed with 4KB-contiguous partition rows; h keeps both experts
stacked on 2x64 partitions so the up matmul contracts all 128 at once.
"""

import os
import sys

sys.path.insert(0, "/opt/trn_rl_repo")

import numpy as np

import concourse.bass as bass
import concourse.mybir as mybir
import concourse.tile as tile

F32 = mybir.dt.float32
F32R = mybir.dt.float32r
F16 = mybir.dt.float16
AF = mybir.ActivationFunctionType
ALU = mybir.AluOpType
ACT_FN = [AF.Gelu]  # overridable for CoreSim (Gelu unimplemented there)

B, S, D, BN, E = 32, 2048, 1024, 64, 8
NCORES = 8
BPC = B // NCORES  # samples per core
NDC = D // 128     # 8 interleaved d-chunks (d = p*8 + j)
NST = S // 512     # 4 s-tiles of 512 (down/gelu)
NSC = S // 128     # 16 s-chunks of 128 (up)


def _split_multiwait(nc):
    """The pinned walrus encodes at most one sync-wait per instruction;
    hoist extra waits into standalone EventSemaphore instructions."""
    fixn = 0
    for f in nc.m.functions:
        for b in f.blocks:
            if not any(
                i.sync_info is not None
                and i.sync_info.on_wait is not None
                and len(i.sync_info.on_wait) > 1
                for i in b.instructions
            ):
                continue
            out = []
            for inst in b.instructions:
                si = inst.sync_info
                if si is not None and si.on_wait is not None and len(si.on_wait) > 1:
                    waits = list(si.on_wait)
                    for w in waits[:-1]:
                        ev = mybir.InstEventSemaphore(
                            name=f"I-mwfix-{fixn}", engine=inst.engine
                        )
                        ev.sync_info = mybir.SyncInfo(on_wait=[w], on_update=[])
                        out.append(ev)
                        fixn += 1
                    inst.sync_info = mybir.SyncInfo(
                        on_wait=[waits[-1]],
                        on_update=list(si.on_update) if si.on_update else [],
                    )
                out.append(inst)
            b.instructions = out
    return fixn


def build_nc(dbg=0):
    """Build the per-core Bass program (SPMD: same program, different shard)."""
    nc = bass.Bass()

    # x arrives fp16, pre-transposed and d-interleaved: xt[b, j, p, s] =
    # x[b, s, p*8+j]; each chunk-j tile is a fully contiguous 512KB DRAM block
    xt_in = nc.dram_tensor("xt", [BPC, NDC, 128, S], F16, kind="ExternalInput")
    # per-sample selected experts' weights, host-gathered and packed:
    # wdu[b, p, 0:1024] = down (j, k*BN+c layout); wdu[b, p, 1024:2048] = up
    # row p = k*BN+c, pre-scaled by the renormalized routing weight
    wdu_in = nc.dram_tensor("wdu", [BPC, 128, 2 * NDC * 2 * BN], F16,
                            kind="ExternalInput")
    # bds[0, b*128 + k*BN+c] = down_b[e_bk, c]
    bds = nc.dram_tensor("bds", [1, BPC * 2 * BN], F32R, kind="ExternalInput")
    ones_in = nc.dram_tensor("ones", [1, 512], F32R, kind="ExternalInput")
    out_t = nc.dram_tensor("out", [BPC, S, D], F16, kind="ExternalOutput")

    with tile.TileContext(nc) as tc:
        with (
            tc.tile_pool(name="singles", bufs=1) as singles,
            tc.tile_pool(name="xt", bufs=8) as xt_p,
            tc.tile_pool(name="ht", bufs=2) as ht_p,
            tc.tile_pool(name="wg", bufs=2) as wg_p,
            tc.tile_pool(name="osb", bufs=3) as osb_p,
            tc.tile_pool(name="hps", bufs=4, space="PSUM") as hps_p,
            tc.tile_pool(name="ops", bufs=2, space="PSUM") as ops_p,
        ):
            # ---- weight + x streams, interleaved per sample so each
            # sample's weights land just before its x tiles. Sample 0 loads x
            # in 8 chunk-tiles (fine-grained: its down matmul chases the
            # stream); later samples use 2 big transfers (fewer ~0.7us DMA
            # trigger instructions, and their data lands ahead of use anyway)
            bds_sb = singles.tile([1, BPC * 2 * BN], F32R, tag="bds")
            nc.sync.dma_start(bds_sb[:], bds[:])
            ones512 = singles.tile([1, 512], F32R, tag="ones")
            nc.sync.dma_start(ones512[:], ones_in[:])
            xt = [[None] * NDC for _ in range(BPC)]
            wdu = []
            for b in range(BPC):
                # even samples stream on the SP queue, odd on the ACT
                # queue (a single queue tops out ~320 GB/s); stores all go
                # through the SWDGE queue so they never queue behind loads
                eng = nc.sync if b % 2 == 0 else nc.scalar
                w = wg_p.tile([128, 2 * NDC * 2 * BN], F16, tag="wdu",
                              name=f"wdu_{b}")
                eng.dma_start(w[:], wdu_in[b, :, :])
                wdu.append(w)
                for j in range(NDC):
                    xt_sb = xt_p.tile([128, S], F16, tag="xt0", bufs=32,
                                      name=f"xt_{b}_{j}")
                    eng.dma_start(xt_sb[:], xt_in[b, j, :, :])
                    xt[b][j] = xt_sb


            # ---- per-sample compute
            for b in range(BPC):
                # down matmul: contract d over 8 chunks; down bias enters as
                # a rank-1 f32r matmul into the same PSUM accumulation group
                h_ps = [
                    hps_p.tile([128, 512], F32, tag="hps", name=f"hps_{b}_{st}")
                    for st in range(NST)
                ]
                for j in range(NDC):
                    xtj = xt[b][j] if b == 0 else xt[b][j]
                    for st in range(NST):
                        nc.tensor.matmul(
                            h_ps[st][:],
                            wdu[b][:, j * 2 * BN:(j + 1) * 2 * BN],
                            xtj[:, st * 512:(st + 1) * 512],
                            start=(j == 0), stop=False,
                        )
                for st in range(NST):
                    nc.tensor.matmul(
                        h_ps[st][:],
                        bds_sb[0:1, b * 128:(b + 1) * 128], ones512[:],
                        start=False, stop=True,
                    )
                ht = ht_p.tile([128, S], F16, tag="ht")
                for st in range(NST):
                    nc.scalar.activation(
                        ht[:, st * 512:(st + 1) * 512], h_ps[st][:], ACT_FN[0]
                    )

                # up matmul (contract stacked 2x64) + PSUM->SBUF fp16 copies
                # split across DVE/ACT; stores batch 4 s-chunks per trigger
                for sq in range(NSC // 4):
                    o_sb = osb_p.tile([128, 4, D], F16, tag="osb")
                    for i in range(4):
                        st = 4 * sq + i
                        o_ps = ops_p.tile([128, D], F32, tag="ops",
                                          name=f"ops_{b}_{st}")
                        for dh in range(2):
                            nc.tensor.matmul(
                                o_ps[:, dh * 512:(dh + 1) * 512],
                                ht[:, st * 128:(st + 1) * 128],
                                wdu[b][:, 1024 + dh * 512:1024 + (dh + 1) * 512],
                                start=True, stop=True,
                            )
                        if st % 8 < 5:
                            nc.vector.tensor_copy(o_sb[:, i, :], o_ps[:])
                        else:
                            nc.scalar.copy(o_sb[:, i, :], o_ps[:])
                    eng = nc.gpsimd
                    eng.dma_start(
                        out_t[b, sq * 512:(sq + 1) * 512, :].rearrange(
                            "(i p) d -> p i d", p=128),
                        o_sb[:],
                    )

    return nc


_NC_CACHE = {}


def _get_nc(dbg=0):
    if ("nc", dbg) not in _NC_CACHE:
        nc = build_nc(dbg)
        _split_multiwait(nc)  # after build: walrus wants <=1 wait per inst
        _NC_CACHE[("nc", dbg)] = nc
    return _NC_CACHE[("nc", dbg)]


def host_routing(pooled, gate_w):
    """The reference routing in exact fp32 numpy."""
    lg = pooled @ gate_w.T
    g = np.exp(lg - lg.max(1, keepdims=True))
    g = g / g.sum(1, keepdims=True)
    ti = np.argsort(-g, axis=1, kind="stable")[:, :2]
    tw = np.take_along_axis(g, ti, 1)
    tw = tw / (tw.sum(1, keepdims=True) + 1e-8)
    return ti, tw


def make_in_maps(x, gate_w, down_w, down_b, up_w, up_b):
    pooled = x.mean(axis=1, dtype=np.float32)  # (B, D) exact f32 routing input
    ti, tw = host_routing(pooled, gate_w)
    # per-sample expert selections in matmul layouts, packed [down | up]
    wd_sel = down_w[ti]                        # (B, 2, BN, D) f32
    wd_sel = wd_sel.transpose(0, 3, 1, 2)      # (B, D, 2, BN)
    wd_sel = wd_sel.reshape(B, 128, NDC * 2 * BN).astype(np.float16)
    wu_sel = (tw[:, :, None, None] * up_w[ti])  # (B, 2, D, BN) scaled
    wu_sel = wu_sel.transpose(0, 1, 3, 2).reshape(B, 2 * BN, D).astype(np.float16)
    wdu = np.concatenate([wd_sel, wu_sel], axis=2)  # (B, 128, 2048)
    bd_sel = down_b[ti].reshape(B, 2 * BN).astype(np.float32)
    in_maps = []
    for c in range(NCORES):
        sl = slice(c * BPC, (c + 1) * BPC)
        m = {
            "xt": np.ascontiguousarray(
                x[sl].reshape(BPC, S, 128, NDC).transpose(0, 3, 2, 1)
                .astype(np.float16)),
            "wdu": np.ascontiguousarray(wdu[sl]),
            "bds": np.ascontiguousarray(bd_sel[sl].reshape(1, BPC * 2 * BN)),
            "ones": np.ones((1, 512), np.float32),
        }
        in_maps.append(m)
    return in_maps, (ti, tw)


def kernel(x, gate_w, down_w, down_b, up_w, up_b, _trace=False, _dbg=0,
           **_ignored):
    from concourse.bass_utils import run_bass_kernel_spmd

    nc = _get_nc(_dbg)
    in_maps, (ti, tw) = make_in_maps(x, gate_w, down_w, down_b, up_w, up_b)
    res = run_bass_kernel_spmd(nc, in_maps, list(range(NCORES)), trace=_trace)
    out = np.concatenate(
        [np.asarray(res.results[c]["out"]) for c in range(NCORES)], axis=0
    ).astype(np.float32)
    # out bias (a routing-weighted combination of up_b rows) is added during
    # the host-side fp16->f32 conversion; the device handles everything else
    bias = (tw[:, :, None] * up_b[ti]).sum(axis=1)
    out += bias[:, None, :]
    if _trace:
        kernel.last_result = res
    return out


# revision 34
# speedup vs baseline: 1.0344x; 1.0344x over previous
"""MoE adapter layer kernel for Trainium2 (8 NeuronCores, data-parallel over B).

Reference computation (per sample b):
    pooled = x[b].mean(axis=0)                       # (D,)
    gate   = softmax(pooled @ gate_w.T)              # (E,)
    top2 values/indices, renormalized weights w0,w1
    h_k    = gelu(x[b] @ Wd[ik].T + bd[ik])          # (S, BN)
    out[b] = sum_k w_k * h_k @ Wu[ik].T + sum_k w_k * bu[ik]

Shapes: B=32, S=2048, D=1024, BN=64, E=8, K=2.

Strategy: shard B over the 8 cores (4 samples each). The device runs the
heavy math -- per sample a (2048x1024)@(1024x128) down matmul, exact
GELU, and a (2048x128)@(128x1024) up matmul -- as one statically
scheduled fp16 pipeline at full PE rate with fp32 PSUM accumulation
(~7e-4 max-rel error). The streamed tensors (x in, out) travel as fp16,
halving HBM traffic vs fp32.

Routing runs on the host in exact fp32 (softmax over 8 gates + top-2 on
the seq-mean -- microseconds of numpy on <40KB of data): the host ships,
per sample, the two selected experts' down/up weights (up pre-scaled by
the renormalized routing weights) in the layouts the matmuls want, plus
the stacked down-bias row. This keeps the device free of dynamic
(register-indexed) DMAs, whose descriptor patching and serialized
trigger chain dominated the kernel's startup latency; every device DMA
is static and prefetchable. The down bias enters as one extra f32r
rank-1 matmul into the same PSUM accumulation group as the down matmul
(bias lives on partitions, expressible as bd_row^T @ ones); the out bias
(a per-sample constant row) is added during the host-side fp16->f32
conversion of the output.

Down matmul contracts D as 8 interleaved chunks (d = p*8+j) so x ships
transposed with 4KB-contiguous partition rows; h keeps both experts
stacked on 2x64 partitions so the up matmul contracts all 128 at once.
"""

import os
import sys

sys.path.insert(0, "/opt/trn_rl_repo")

import numpy as np

import concourse.bass as bass
import concourse.mybir as mybir
import concourse.tile as tile

F32 = mybir.dt.float32
F32R = mybir.dt.float32r
F16 = mybir.dt.float16
AF = mybir.ActivationFunctionType
ALU = mybir.AluOpType
ACT_FN = [AF.Gelu]  # overridable for CoreSim (Gelu unimplemented there)

B, S, D, BN, E = 32, 2048, 1024, 64, 8
NCORES = 8
BPC = B // NCORES  # samples per core
NDC = D // 128     # 8 interleaved d-chunks (d = p*8 + j)
NST = S // 512     # 4 s-tiles of 512 (down/gelu)
NSC = S // 128     # 16 s-chunks of 128 (up)


def _split_multiwait(nc):
    """The pinned walrus encodes at most one sync-wait per instruction;
    hoist extra waits into standalone EventSemaphore instructions."""
    fixn = 0
    for f in nc.m.functions:
        for b in f.blocks:
            if not any(
                i.sync_info is not None
                and i.sync_info.on_wait is not None
                and len(i.sync_info.on_wait) > 1
                for i in b.instructions
            ):
                continue
            out = []
            for inst in b.instructions:
                si = inst.sync_info
                if si is not None and si.on_wait is not None and len(si.on_wait) > 1:
                    waits = list(si.on_wait)
                    for w in waits[:-1]:
                        ev = mybir.InstEventSemaphore(
                            name=f"I-mwfix-{fixn}", engine=inst.engine
                        )
                        ev.sync_info = mybir.SyncInfo(on_wait=[w], on_update=[])
                        out.append(ev)
                        fixn += 1
                    inst.sync_info = mybir.SyncInfo(
                        on_wait=[waits[-1]],
                        on_update=list(si.on_update) if si.on_update else [],
                    )
                out.append(inst)
            b.instructions = out
    return fixn


def build_nc(dbg=0):
    """Build the per-core Bass program (SPMD: same program, different shard)."""
    nc = bass.Bass()

    # x arrives fp16, pre-transposed and d-interleaved: xt[b, j, p, s] =
    # x[b, s, p*8+j]; each chunk-j tile is a fully contiguous 512KB DRAM block
    xt_in = nc.dram_tensor("xt", [BPC, NDC, 128, S], F16, kind="ExternalInput")
    # per-sample selected experts' weights, host-gathered and packed:
    # wdu[b, p, 0:1024] = down (j, k*BN+c layout); wdu[b, p, 1024:2048] = up
    # row p = k*BN+c, pre-scaled by the renormalized routing weight
    wdu_in = nc.dram_tensor("wdu", [BPC, 128, 2 * NDC * 2 * BN], F16,
                            kind="ExternalInput")
    # bds[0, b*128 + k*BN+c] = down_b[e_bk, c]
    bds = nc.dram_tensor("bds", [1, BPC * 2 * BN], F32R, kind="ExternalInput")
    ones_in = nc.dram_tensor("ones", [1, 512], F32R, kind="ExternalInput")
    out_t = nc.dram_tensor("out", [BPC, S, D], F16, kind="ExternalOutput")

    with tile.TileContext(nc) as tc:
        with (
            tc.tile_pool(name="singles", bufs=1) as singles,
            tc.tile_pool(name="xt", bufs=8) as xt_p,
            tc.tile_pool(name="ht", bufs=2) as ht_p,
            tc.tile_pool(name="wg", bufs=2) as wg_p,
            tc.tile_pool(name="osb", bufs=3) as osb_p,
            tc.tile_pool(name="hps", bufs=4, space="PSUM") as hps_p,
            tc.tile_pool(name="ops", bufs=2, space="PSUM") as ops_p,
        ):
            # ---- weight + x streams, interleaved per sample so each
            # sample's weights land just before its x tiles. Sample 0 loads x
            # in 8 chunk-tiles (fine-grained: its down matmul chases the
            # stream); later samples use 2 big transfers (fewer ~0.7us DMA
            # trigger instructions, and their data lands ahead of use anyway)
            bds_sb = singles.tile([1, BPC * 2 * BN], F32R, tag="bds")
            nc.sync.dma_start(bds_sb[:], bds[:])
            ones512 = singles.tile([1, 512], F32R, tag="ones")
            nc.sync.dma_start(ones512[:], ones_in[:])
            xt = [[None] * NDC for _ in range(BPC)]
            wdu = []
            for b in range(BPC):
                # the whole in-stream stays on ONE queue: sequential
                # 512KB blocks keep DRAM reads row-local (~400 GB/s); a
                # second read queue interleaves pages and loses ~25%
                w = wg_p.tile([128, 2 * NDC * 2 * BN], F16, tag="wdu",
                              name=f"wdu_{b}")
                nc.sync.dma_start(w[:], wdu_in[b, :, :])
                wdu.append(w)
                for j in range(NDC):
                    xt_sb = xt_p.tile([128, S], F16, tag="xt0", bufs=32,
                                      name=f"xt_{b}_{j}")
                    nc.sync.dma_start(xt_sb[:], xt_in[b, j, :, :])
                    xt[b][j] = xt_sb


            # ---- per-sample compute
            for b in range(BPC):
                # down matmul: contract d over 8 chunks; down bias enters as
                # a rank-1 f32r matmul into the same PSUM accumulation group
                h_ps = [
                    hps_p.tile([128, 512], F32, tag="hps", name=f"hps_{b}_{st}")
                    for st in range(NST)
                ]
                for j in range(NDC):
                    xtj = xt[b][j] if b == 0 else xt[b][j]
                    for st in range(NST):
                        nc.tensor.matmul(
                            h_ps[st][:],
                            wdu[b][:, j * 2 * BN:(j + 1) * 2 * BN],
                            xtj[:, st * 512:(st + 1) * 512],
                            start=(j == 0), stop=False,
                        )
                for st in range(NST):
                    nc.tensor.matmul(
                        h_ps[st][:],
                        bds_sb[0:1, b * 128:(b + 1) * 128], ones512[:],
                        start=False, stop=True,
                    )
                ht = ht_p.tile([128, S], F16, tag="ht")
                for st in range(NST):
                    nc.scalar.activation(
                        ht[:, st * 512:(st + 1) * 512], h_ps[st][:], ACT_FN[0]
                    )

                # up matmul (contract stacked 2x64) + PSUM->SBUF fp16 copies
                # split across DVE/ACT; stores batch 4 s-chunks per trigger
                for sq in range(NSC // 4):
                    o_sb = osb_p.tile([128, 4, D], F16, tag="osb")
                    for i in range(4):
                        st = 4 * sq + i
                        o_ps = ops_p.tile([128, D], F32, tag="ops",
                                          name=f"ops_{b}_{st}")
                        for dh in range(2):
                            nc.tensor.matmul(
                                o_ps[:, dh * 512:(dh + 1) * 512],
                                ht[:, st * 128:(st + 1) * 128],
                                wdu[b][:, 1024 + dh * 512:1024 + (dh + 1) * 512],
                                start=True, stop=True,
                            )
                        if st % 8 < 5:
                            nc.vector.tensor_copy(o_sb[:, i, :], o_ps[:])
                        else:
                            nc.scalar.copy(o_sb[:, i, :], o_ps[:])
                    eng = nc.scalar if sq % 2 == 0 else nc.gpsimd
                    eng.dma_start(
                        out_t[b, sq * 512:(sq + 1) * 512, :].rearrange(
                            "(i p) d -> p i d", p=128),
                        o_sb[:],
                    )

    return nc


_NC_CACHE = {}


def _get_nc(dbg=0):
    if ("nc", dbg) not in _NC_CACHE:
        nc = build_nc(dbg)
        _split_multiwait(nc)  # after build: walrus wants <=1 wait per inst
        _NC_CACHE[("nc", dbg)] = nc
    return _NC_CACHE[("nc", dbg)]


def host_routing(pooled, gate_w):
    """The reference routing in exact fp32 numpy."""
    lg = pooled @ gate_w.T
    g = np.exp(lg - lg.max(1, keepdims=True))
    g = g / g.sum(1, keepdims=True)
    ti = np.argsort(-g, axis=1, kind="stable")[:, :2]
    tw = np.take_along_axis(g, ti, 1)
    tw = tw / (tw.sum(1, keepdims=True) + 1e-8)
    return ti, tw


def make_in_maps(x, gate_w, down_w, down_b, up_w, up_b):
    pooled = x.mean(axis=1, dtype=np.float32)  # (B, D) exact f32 routing input
    ti, tw = host_routing(pooled, gate_w)
    # per-sample expert selections in matmul layouts, packed [down | up]
    wd_sel = down_w[ti]                        # (B, 2, BN, D) f32
    wd_sel = wd_sel.transpose(0, 3, 1, 2)      # (B, D, 2, BN)
    wd_sel = wd_sel.reshape(B, 128, NDC * 2 * BN).astype(np.float16)
    wu_sel = (tw[:, :, None, None] * up_w[ti])  # (B, 2, D, BN) scaled
    wu_sel = wu_sel.transpose(0, 1, 3, 2).reshape(B, 2 * BN, D).astype(np.float16)
    wdu = np.concatenate([wd_sel, wu_sel], axis=2)  # (B, 128, 2048)
    bd_sel = down_b[ti].reshape(B, 2 * BN).astype(np.float32)
    in_maps = []
    for c in range(NCORES):
        sl = slice(c * BPC, (c + 1) * BPC)
        m = {
            "xt": np.ascontiguousarray(
                x[sl].reshape(BPC, S, 128, NDC).transpose(0, 3, 2, 1)
                .astype(np.float16)),
            "wdu": np.ascontiguousarray(wdu[sl]),
            "bds": np.ascontiguousarray(bd_sel[sl].reshape(1, BPC * 2 * BN)),
            "ones": np.ones((1, 512), np.float32),
        }
        in_maps.append(m)
    return in_maps, (ti, tw)


def kernel(x, gate_w, down_w, down_b, up_w, up_b, _trace=False, _dbg=0,
           **_ignored):
    from concourse.bass_utils import run_bass_kernel_spmd

    nc = _get_nc(_dbg)
    in_maps, (ti, tw) = make_in_maps(x, gate_w, down_w, down_b, up_w, up_b)
    res = run_bass_kernel_spmd(nc, in_maps, list(range(NCORES)), trace=_trace)
    out = np.concatenate(
        [np.asarray(res.results[c]["out"]) for c in range(NCORES)], axis=0
    ).astype(np.float32)
    # out bias (a routing-weighted combination of up_b rows) is added during
    # the host-side fp16->f32 conversion; the device handles everything else
    bias = (tw[:, :, None] * up_b[ti]).sum(axis=1)
    out += bias[:, None, :]
    if _trace:
        kernel.last_result = res
    return out
